# revision 1
# baseline (speedup 1.0000x reference)
"""HGNN conv kernel for Trainium2, 8 NeuronCores.

out = dv ⊙ (H @ (W·de ⊙ (H^T @ (dv ⊙ (x@weight))))) + bias
  dv = rowsum(H)^-1/2  [N], de = colsum(H)^-1  [E]
  N=16384, E=8192, F=64.

Sharding: H/x row-sharded over N across 8 cores (2048 rows each).
Host preps per-core fp8(e4m3) H shards in both layouts, paired for the
tensor engine's DoubleRow mode (two 128-row chunks interleaved at the
j level) — pure layout/precision transforms; all FLOPs run on device.
fp8 is numerically safe here: the output is dominated by a coherent DC
path through the all-positive H (validated: rel err ~1.4e-3 ≪ 2e-2).
All matmul operand pairs share one dtype (mixed-dtype matmul and
tensor_tensor_reduce are hardware faults on this part - found the hard
way). Power-of-2 scales keep the fp8 operands in normal range: weight
is pre-scaled x64 (so xs=dv*xw*64), W x16 (so y2 is stored x1024), and
the final per-row scale is dv/1024.

Device per core:
  prep: xw for all 16 n-tiles into one PSUM strip. Per n-tile rowsum
        split across DVE (front half) + ACT accum (back half), summed.
  pass1: two 4-pair groups; per 512-col e-block one PSUM bank takes the
        full DoubleRow chain (2 banks ping-pong; DVE copy/add drains)
        -> ybuf bf16; half-wise AllReduce (bf16) kicks when final.
  y2:   XBAR dma-transpose of the reduced half -> [128e, 32, 64]; de
        from the ones-column strip; ACT scales by 16*W*de -> y2 e4m3.
  pass2: stream HT pair-supertiles [128, 2, 2048]; 4 persistent PSUM
        banks accumulate out^T via DoubleRow over 32 pairs; finish via
        PE transpose + ACT dv/1024 scale + DVE bias add.

Measured 281,410 ns (vs 437,065 ns baseline). Remaining bottlenecks,
in order: (1) the two 0.53MB bf16 AllReduces run at only 10-18 GB/s
effective and the first absorbs all inter-core skew (34-53 ns noise
run-to-run) -> ~70us exposed latency; (2) the rowsum prep phase is
DVE+ACT lane-bound (~100us each; fp8 reads get no 2x vector mode). A
third rowsum lane via gpsimd.tensor_tensor fold (fp8 in -> f16 out,
then DVE f16 reduce) is HARDWARE-VALIDATED EXACT in a micro-test and
worth ~15-25us; (3) effective HBM bandwidth measures ~207 GB/s/core
vs 358 spec (mbu ~23%) - the biggest unexplained gap.
"""

import numpy as np
import ml_dtypes

N, E, F = 16384, 8192, 64
NCORES = 8
NL = N // NCORES          # 2048 rows per core
P = 128
NT = NL // P              # 16 n-tiles per core
NPAIR = NT // 2           # 8 natural pair-supertiles
ET = E // P               # 64 e-tiles
EPAIR = ET // 2           # 32 transposed pair-supertiles
EBLK = 512
NB_BLK = 16               # pass-1 e-blocks of 512
HE = E // 2               # collective half width
G = 4                     # pairs per pass-1 psum chain group
NBLK = 512
NB = NL // NBLK           # 4 pass-2 output blocks
XPAD = 80                 # padded xs/y2 row length (16-aligned)

_prog_cache = {}


def _build_program():
    import concourse.bass as bass
    import concourse.mybir as mybir
    import concourse.tile as tile
    from concourse import bacc
    from concourse.masks import make_identity

    f32 = mybir.dt.float32
    bf16 = mybir.dt.bfloat16
    f8 = mybir.dt.float8e4
    DR = mybir.MatmulPerfMode.DoubleRow
    Copy = mybir.ActivationFunctionType.Copy
    add = mybir.AluOpType.add
    mult = mybir.AluOpType.mult
    X = mybir.AxisListType.X

    nc = bacc.Bacc(
        "TRN2", target_bir_lowering=False, debug=False, num_devices=NCORES
    )
    h = nc.declare_dram_parameter("h", [NPAIR, P, 2, E], f8, isOutput=False)
    ht = nc.declare_dram_parameter("ht", [EPAIR, P, 2, NL], f8, isOutput=False)
    xt = nc.declare_dram_parameter("xt", [F, NL], f32, isOutput=False)
    wmat = nc.declare_dram_parameter("wmat", [F, F], f32, isOutput=False)
    wstr = nc.declare_dram_parameter("wstr", [P, ET], f32, isOutput=False)
    biasb = nc.declare_dram_parameter("biasb", [P, NB, F], f32, isOutput=False)
    out = nc.declare_dram_parameter("out", [NL, F], f32, isOutput=True)

    with tile.TileContext(nc) as tc:
        with (
            tc.tile_pool(name="hp", bufs=NPAIR) as hp,           # resident H pairs
            tc.tile_pool(name="smallp", bufs=1) as smallp,       # persistent small
            tc.tile_pool(name="rp", bufs=4) as rp,               # rowsum temps
            tc.tile_pool(name="ybp", bufs=1) as ybp,             # ybuf halves
            tc.tile_pool(name="y2rp", bufs=1) as y2rp,           # y2 raw staging
            tc.tile_pool(name="finp", bufs=2) as finp,           # out staging
            tc.tile_pool(name="ps_xw", bufs=1, space="PSUM") as ps_xw,
            tc.tile_pool(name="ps_y", bufs=2, space="PSUM") as ps_y,
            tc.tile_pool(name="ps_o", bufs=1, space="PSUM") as ps_o,
            tc.tile_pool(name="dramp", bufs=1, space="DRAM") as dramp,
        ):
            # ---- persistent small tensors ----
            xt_sb = smallp.tile([F, NL], f32, tag="xt")
            nc.sync.dma_start(xt_sb[:], xt[:, :])
            wmat_sb = smallp.tile([F, F], f32, tag="wmat")
            nc.sync.dma_start(wmat_sb[:], wmat[:, :])
            wstr_sb = smallp.tile([P, ET], f32, tag="wstr")
            nc.sync.dma_start(wstr_sb[:], wstr[:, :])
            bias_sb = smallp.tile([P, NB, F], f32, tag="bias")
            nc.sync.dma_start(bias_sb[:], biasb[:, :, :])
            dv_all = smallp.tile([P, NT], f32, tag="dv")
            dvf = smallp.tile([P, NT], f32, tag="dvf")
            xs_sb = smallp.tile([P, NPAIR, 2, XPAD], f8, tag="xs")
            nc.gpsimd.memset(xs_sb[:, :, :, F:F + 1], 1.0)
            y2_sb = smallp.tile([P, EPAIR, 2, XPAD], f8, tag="y2")
            dumA = smallp.tile([P, E // 2], f8, tag="dumA")  # ACT rowsum sink
            ident = smallp.tile([F, F], f32, tag="ident")
            make_identity(nc, ident)

            # ---- xw = x @ weight (x64) for all n-tiles, kept in PSUM ----
            xw_all = ps_xw.tile([P, NT * F], f32, tag="xw")
            for t in range(NT):
                nc.tensor.matmul(
                    xw_all[:, t * F:(t + 1) * F],
                    lhsT=xt_sb[:, t * P:(t + 1) * P], rhs=wmat_sb[:],
                    start=True, stop=True,
                )

            h_tiles = []

            def prep_pair(s):
                h_s = hp.tile([P, 2, E], f8, tag="h")
                eng = nc.sync if s % 2 == 0 else nc.scalar
                eng.dma_start(h_s[:], h[s, :, :, :])
                h_tiles.append(h_s)
                for j in range(2):
                    t = 2 * s + j
                    # rowsum split: DVE front half, ACT-accum back half
                    r0 = rp.tile([P, 1], f32, tag="r0")
                    nc.vector.tensor_reduce(
                        out=r0[:], in_=h_s[:, j, 0:E // 2], axis=X, op=add
                    )
                    r1 = rp.tile([P, 1], f32, tag="r1")
                    nc.scalar.activation(
                        out=dumA[:], in_=h_s[:, j, E // 2:E], func=Copy,
                        accum_out=r1[:],
                    )
                    rinv = rp.tile([P, 1], f32, tag="rinv")
                    nc.vector.tensor_tensor(out=rinv[:], in0=r0[:], in1=r1[:], op=add)
                    nc.vector.reciprocal(out=rinv[:], in_=rinv[:])
                    nc.scalar.sqrt(out=dv_all[:, t:t + 1], in_=rinv[:])
                    nc.scalar.activation(
                        out=xs_sb[:, s, j, 0:F], in_=xw_all[:, t * F:(t + 1) * F],
                        func=Copy, scale=dv_all[:, t:t + 1],
                    )

            for s in range(G):
                prep_pair(s)

            # ---- pass 1 (DoubleRow chains, 2 rotating banks) ----
            ybufs = [ybp.tile([F + 1, HE], bf16, name=f"yb{q}") for q in range(2)]

            def p1_block(b, g):
                yps = ps_y.tile([F + 1, EBLK], f32, tag="yps")
                for i in range(G):
                    pr = g * G + i
                    nc.tensor.matmul(
                        yps[:], lhsT=xs_sb[:, pr, :, 0:F + 1],
                        rhs=h_tiles[pr][:, :, b * EBLK:(b + 1) * EBLK],
                        start=(i == 0), stop=(i == G - 1),
                        perf_mode=DR,
                    )
                return yps

            def kick_half(q):
                b_in = dramp.tile([F + 1, HE], bf16, name=f"bi{q}")
                b_out = dramp.tile(
                    [F + 1, HE], bf16, name=f"bo{q}", addr_space="Shared"
                )
                nc.scalar.dma_start(b_in[:], ybufs[q][:])
                nc.gpsimd.collective_compute(
                    "AllReduce",
                    add,
                    ins=[b_in[:].opt()],
                    outs=[b_out[:].opt()],
                    replica_groups=[list(range(NCORES))],
                )
                # y2 half prep: XBAR transpose + de/wde + scale
                y2raw = y2rp.tile([P, HE // P, F], bf16, tag="y2r")
                nc.scalar.dma_start_transpose(y2raw[:], b_out[0:F, :])
                cs = smallp.tile([P, HE // P], bf16, name=f"cs{q}")
                nc.scalar.dma_start(
                    cs[:], b_out[F, :].rearrange("(o p) -> p o", p=P)
                )
                de_t = smallp.tile([P, HE // P], f32, name=f"de{q}")
                nc.vector.reciprocal(out=de_t[:], in_=cs[:])
                wde = smallp.tile([P, HE // P], f32, name=f"wde{q}")
                nc.vector.tensor_tensor(
                    out=wde[:], in0=de_t[:],
                    in1=wstr_sb[:, q * (HE // P):(q + 1) * (HE // P)], op=mult,
                )
                for c in range(HE // P):
                    o = q * (HE // P) + c
                    nc.scalar.activation(
                        out=y2_sb[:, o // 2, o % 2, 0:F], in_=y2raw[:, c, :],
                        func=Copy, scale=wde[:, c:c + 1],
                    )

            # phase A: group 0 over blocks 0..7 (prep of pairs 4..7 interleaves)
            for b in range(8):
                if b % 2 == 0:
                    prep_pair(G + b // 2)
                yps = p1_block(b, 0)
                nc.vector.tensor_copy(
                    out=ybufs[0][:, b * EBLK:(b + 1) * EBLK], in_=yps[:]
                )
            # phase B: group 1 over blocks 0..7 -> half 0 final, collective flies
            for b in range(8):
                yps = p1_block(b, 1)
                dst = ybufs[0][:, b * EBLK:(b + 1) * EBLK]
                nc.vector.tensor_tensor(out=dst, in0=dst, in1=yps[:], op=add)
            kick_half(0)
            # phase C/D: blocks 8..15 while half-0 AllReduce is in flight
            for b in range(8, NB_BLK):
                yps = p1_block(b, 0)
                nc.vector.tensor_copy(
                    out=ybufs[1][:, (b - 8) * EBLK:(b - 7) * EBLK], in_=yps[:]
                )
            for b in range(8, NB_BLK):
                yps = p1_block(b, 1)
                dst = ybufs[1][:, (b - 8) * EBLK:(b - 7) * EBLK]
                nc.vector.tensor_tensor(out=dst, in0=dst, in1=yps[:], op=add)
            kick_half(1)

            # ---- pass 2: out^T accumulation via DoubleRow over e-pairs ----
            # HT streamed as 8x 4-supertile chunks into recycled hp slots.
            chunks = []
            for c in range(EPAIR // 4):
                ck = hp.tile([P, 4, 2, NL], f8, tag="h")
                eng = nc.sync if c % 2 == 0 else nc.scalar
                eng.dma_start(
                    ck[:],
                    ht[c * 4:(c + 1) * 4, :, :, :].rearrange("u p j n -> p u j n"),
                )
                chunks.append(ck)
            o_tiles = [ps_o.tile([F, NBLK], f32, name=f"o{j}") for j in range(NB)]
            for u in range(EPAIR):
                htt = chunks[u // 4]
                for j in range(NB):
                    nc.tensor.matmul(
                        o_tiles[j][:], lhsT=y2_sb[:, u, :, 0:F],
                        rhs=htt[:, u % 4, :, j * NBLK:(j + 1) * NBLK],
                        start=(u == 0), stop=(u == EPAIR - 1),
                        perf_mode=DR,
                    )

            # ---- finish: transpose back, dv/1024 scale, bias, store ----
            nc.vector.tensor_scalar_mul(dvf[:], dv_all[:], 1.0 / 1024.0)
            for j in range(NB):
                s1 = finp.tile([F, NBLK], f32, tag="s1")
                nc.vector.tensor_copy(out=s1[:], in_=o_tiles[j][:])
                osb = finp.tile([P, NBLK // P, F], f32, tag="osb")
                for c in range(NBLK // P):
                    tp = ps_y.tile([P, F], f32, tag="yps")
                    nc.tensor.transpose(
                        tp[:], s1[:, c * P:(c + 1) * P], ident[:]
                    )
                    nc.scalar.activation(
                        out=osb[:, c, :], in_=tp[:], func=Copy,
                        scale=dvf[:, j * (NBLK // P) + c:j * (NBLK // P) + c + 1],
                    )
                nc.vector.tensor_tensor(
                    out=osb[:], in0=osb[:], in1=bias_sb[:], op=add
                )
                nc.scalar.dma_start(
                    out[j * NBLK:(j + 1) * NBLK, :].rearrange(
                        "(c p) f -> p c f", p=P
                    ),
                    osb[:],
                )

    nc.finalize()
    return nc


def _get_program():
    if "nc" not in _prog_cache:
        _prog_cache["nc"] = _build_program()
    return _prog_cache["nc"]


def make_in_maps(x, H, W, weight, bias):
    x = np.asarray(x, dtype=np.float32)
    H = np.asarray(H, dtype=np.float32)
    W = np.asarray(W, dtype=np.float32)
    weight = np.asarray(weight, dtype=np.float32)
    bias = np.asarray(bias, dtype=np.float32)

    H_f8 = H.astype(ml_dtypes.float8_e4m3)
    wstr = np.ascontiguousarray((16.0 * W).reshape(ET, P).T.astype(np.float32))
    biasb = np.ascontiguousarray(
        np.tile(bias[None, None, :], (P, NB, 1)).astype(np.float32)
    )
    wmat = np.ascontiguousarray(64.0 * weight)

    in_maps = []
    for c in range(NCORES):
        hs = H_f8[c * NL:(c + 1) * NL, :]
        # natural pairs: [NPAIR, P, 2, E], (s, p, j) -> row 256 s + 128 j + p
        hpair = np.ascontiguousarray(
            hs.reshape(NPAIR, 2, P, E).transpose(0, 2, 1, 3)
        )
        # transposed pairs: [EPAIR, P, 2, NL], (u, p, j) -> col 256 u + 128 j + p
        htpair = np.ascontiguousarray(
            hs.T.reshape(EPAIR, 2, P, NL).transpose(0, 2, 1, 3)
        )
        in_maps.append({
            "h": hpair,
            "ht": htpair,
            "xt": np.ascontiguousarray(x[c * NL:(c + 1) * NL, :].T),
            "wmat": wmat,
            "wstr": wstr,
            "biasb": biasb,
        })
    return in_maps


def run(x, H, W, weight, bias, trace=False, **kw):
    from concourse.bass_utils import run_bass_kernel_spmd

    nc = _get_program()
    in_maps = make_in_maps(x, H, W, weight, bias)
    res = run_bass_kernel_spmd(nc, in_maps, list(range(NCORES)), trace=trace, **kw)
    out = np.concatenate(
        [res.results[c]["out"] for c in range(NCORES)], axis=0
    ).astype(np.float32)
    return out, res


def kernel(x, H, W, weight, bias):
    out, _ = run(x, H, W, weight, bias, trace=False)
    return out



# revision 7
# speedup vs baseline: 1.0121x; 1.0121x over previous
"""HGNN conv kernel for Trainium2, 8 NeuronCores.

out = dv ⊙ (H @ (W·de ⊙ (H^T @ (dv ⊙ (x@weight))))) + bias
  dv = rowsum(H)^-1/2  [N], de = colsum(H)^-1  [E]
  N=16384, E=8192, F=64.

Sharding: H/x row-sharded over N across 8 cores (2048 rows each).
Host preps per-core fp8(e4m3) H shards in both layouts, paired for the
tensor engine's DoubleRow mode (two 128-row chunks interleaved at the
j level) — pure layout/precision transforms; all FLOPs run on device.
fp8 is numerically safe here: the output is dominated by a coherent DC
path through the all-positive H (validated: rel err ~1.4e-3 ≪ 2e-2).
All matmul operand pairs share one dtype (mixed-dtype matmul and
tensor_tensor_reduce are hardware faults on this part - found the hard
way). Power-of-2 scales keep the fp8 operands in normal range: weight
is pre-scaled x64 (so xs=dv*xw*64), W x16 (so y2 is stored x1024), and
the final per-row scale is dv/1024.

Device per core:
  prep: xw for all 16 n-tiles into one PSUM strip. Per n-tile rowsum
        split across DVE (front half) + ACT accum (back half), summed.
  pass1: two 4-pair groups; per 512-col e-block one PSUM bank takes the
        full DoubleRow chain (2 banks ping-pong; DVE copy/add drains)
        -> ybuf bf16; half-wise AllReduce (bf16) kicks when final.
  y2:   XBAR dma-transpose of the reduced half -> [128e, 32, 64]; de
        from the ones-column strip; ACT scales by 16*W*de -> y2 e4m3.
  pass2: stream HT pair-supertiles [128, 2, 2048]; 4 persistent PSUM
        banks accumulate out^T via DoubleRow over 32 pairs; finish via
        PE transpose + ACT dv/1024 scale + DVE bias add.

v2 changes (from trace analysis of the 316us baseline run):
(1) rowsum prep now runs on THREE lanes per tile: DVE fp8 fold
    (tensor_tensor fp8+fp8 -> f16, then f16 reduce), ACT copy+accum
    mid slice, gpsimd fold (fp8 -> f16) + DVE f16 reduce for the back
    4096. Was DVE+ACT two-lane at ~5.3us/tile each, pegging both
    engines 40-120us and delaying the first AllReduce trigger to
    148us.
(2) pass 2 is split per collective half: u=0..15 only needs y2 of
    half 0 (ready right after AR0), so those 64 matmuls now overlap
    AR1's flight instead of waiting for it (was: pass-2 start 240us).
(3) ht pair-supertile chunks load on the gpsimd/SWDGE queue, issued
    right after the hp slots they reuse are released (chunks 0-3
    after phase C, 4-7 after phase D), keeping sync/scalar HWDGE
    queues free for the collective bounce buffers and y2 transposes.
"""

import numpy as np
import ml_dtypes

N, E, F = 16384, 8192, 64
NCORES = 8
NL = N // NCORES          # 2048 rows per core
P = 128
NT = NL // P              # 16 n-tiles per core
NPAIR = NT // 2           # 8 natural pair-supertiles
ET = E // P               # 64 e-tiles
EPAIR = ET // 2           # 32 transposed pair-supertiles
EBLK = 512
NB_BLK = 16               # pass-1 e-blocks of 512
HE = E // 2               # collective half width
G = 4                     # pairs per pass-1 psum chain group
NBLK = 512
NB = NL // NBLK           # 4 pass-2 output blocks
XPAD = 80                 # padded xs/y2 row length (16-aligned)

_prog_cache = {}


def _build_program():
    import concourse.bass as bass
    import concourse.mybir as mybir
    import concourse.tile as tile
    from concourse import bacc
    from concourse.masks import make_identity

    f32 = mybir.dt.float32
    bf16 = mybir.dt.bfloat16
    f16 = mybir.dt.float16
    f8 = mybir.dt.float8e4
    DR = mybir.MatmulPerfMode.DoubleRow
    Copy = mybir.ActivationFunctionType.Copy
    add = mybir.AluOpType.add
    mult = mybir.AluOpType.mult
    X = mybir.AxisListType.X

    nc = bacc.Bacc(
        "TRN2", target_bir_lowering=False, debug=False, num_devices=NCORES
    )
    h = nc.declare_dram_parameter("h", [NPAIR, P, 2, E], f8, isOutput=False)
    ht = nc.declare_dram_parameter("ht", [EPAIR, P, 2, NL], f8, isOutput=False)
    xt = nc.declare_dram_parameter("xt", [F, NL], f32, isOutput=False)
    wmat = nc.declare_dram_parameter("wmat", [F, F], f32, isOutput=False)
    wstr = nc.declare_dram_parameter("wstr", [P, ET], f32, isOutput=False)
    biasb = nc.declare_dram_parameter("biasb", [P, NB, F], f32, isOutput=False)
    out = nc.declare_dram_parameter("out", [NL, F], f32, isOutput=True)

    with tile.TileContext(nc) as tc:
        with (
            tc.tile_pool(name="hp", bufs=NPAIR) as hp,           # resident H pairs
            tc.tile_pool(name="smallp", bufs=1) as smallp,       # persistent small
            tc.tile_pool(name="rp", bufs=4) as rp,               # rowsum temps
            tc.tile_pool(name="gfp", bufs=2) as gfp,             # gpsimd fold out
            tc.tile_pool(name="dfp", bufs=2) as dfp,             # DVE fold out
            tc.tile_pool(name="ybp", bufs=1) as ybp,             # ybuf halves
            tc.tile_pool(name="y2rp", bufs=1) as y2rp,           # y2 raw staging
            tc.tile_pool(name="finp", bufs=2) as finp,           # out staging
            tc.tile_pool(name="ps_xw", bufs=1, space="PSUM") as ps_xw,
            tc.tile_pool(name="ps_y", bufs=2, space="PSUM") as ps_y,
            tc.tile_pool(name="ps_o", bufs=1, space="PSUM") as ps_o,
            tc.tile_pool(name="dramp", bufs=1, space="DRAM") as dramp,
        ):
            # ---- persistent small tensors ----
            xt_sb = smallp.tile([F, NL], f32, tag="xt")
            nc.sync.dma_start(xt_sb[:], xt[:, :])
            wmat_sb = smallp.tile([F, F], f32, tag="wmat")
            nc.sync.dma_start(wmat_sb[:], wmat[:, :])
            wstr_sb = smallp.tile([P, ET], f32, tag="wstr")
            nc.sync.dma_start(wstr_sb[:], wstr[:, :])
            bias_sb = smallp.tile([P, NB, F], f32, tag="bias")
            nc.sync.dma_start(bias_sb[:], biasb[:, :, :])
            dv_all = smallp.tile([P, NT], f32, tag="dv")
            dvf = smallp.tile([P, NT], f32, tag="dvf")
            xs_sb = smallp.tile([P, NPAIR, 2, XPAD], f8, tag="xs")
            nc.gpsimd.memset(xs_sb[:, :, :, F:F + 1], 1.0)
            y2_sb = smallp.tile([P, EPAIR, 2, XPAD], f8, tag="y2")
            dumA = smallp.tile([P, 2048], f8, tag="dumA")  # ACT rowsum sink
            ident = smallp.tile([F, F], f32, tag="ident")
            make_identity(nc, ident)

            # ---- xw = x @ weight (x64) for all n-tiles, kept in PSUM ----
            xw_all = ps_xw.tile([P, NT * F], f32, tag="xw")
            for t in range(NT):
                nc.tensor.matmul(
                    xw_all[:, t * F:(t + 1) * F],
                    lhsT=xt_sb[:, t * P:(t + 1) * P], rhs=wmat_sb[:],
                    start=True, stop=True,
                )

            h_tiles = []

            def prep_pair(s):
                h_s = hp.tile([P, 2, E], f8, tag="h")
                eng = nc.sync if s % 2 == 0 else nc.scalar
                eng.dma_start(h_s[:], h[s, :, :, :])
                h_tiles.append(h_s)
                for j in range(2):
                    t = 2 * s + j
                    # rowsum 3-lane split:
                    #  DVE fp8-fold [0:2048) -> f16 [P,1024] -> f16 reduce
                    #  ACT copy+accum [2048:4096)
                    #  gpsimd fp8-fold [4096:8192) -> f16 [P,2048] -> DVE
                    dfo = dfp.tile([P, 1024], f16, tag="df")
                    nc.vector.tensor_tensor(
                        out=dfo[:], in0=h_s[:, j, 0:1024],
                        in1=h_s[:, j, 1024:2048], op=add,
                    )
                    gfo = gfp.tile([P, 2048], f16, tag="gf")
                    nc.gpsimd.tensor_tensor(
                        out=gfo[:], in0=h_s[:, j, 4096:6144],
                        in1=h_s[:, j, 6144:8192], op=add,
                    )
                    r0 = rp.tile([P, 1], f32, tag="r0")
                    nc.vector.tensor_reduce(
                        out=r0[:], in_=dfo[:], axis=X, op=add
                    )
                    r1 = rp.tile([P, 1], f32, tag="r1")
                    nc.scalar.activation(
                        out=dumA[:], in_=h_s[:, j, 2048:4096], func=Copy,
                        accum_out=r1[:],
                    )
                    r2 = rp.tile([P, 1], f32, tag="r2")
                    nc.vector.tensor_reduce(
                        out=r2[:], in_=gfo[:], axis=X, op=add
                    )
                    rinv = rp.tile([P, 1], f32, tag="rinv")
                    nc.vector.tensor_tensor(out=rinv[:], in0=r0[:], in1=r1[:], op=add)
                    nc.vector.tensor_tensor(out=rinv[:], in0=rinv[:], in1=r2[:], op=add)
                    nc.vector.reciprocal(out=rinv[:], in_=rinv[:])
                    nc.scalar.sqrt(out=dv_all[:, t:t + 1], in_=rinv[:])
                    nc.scalar.activation(
                        out=xs_sb[:, s, j, 0:F], in_=xw_all[:, t * F:(t + 1) * F],
                        func=Copy, scale=dv_all[:, t:t + 1],
                    )

            for s in range(G):
                prep_pair(s)

            # ---- pass 1 (DoubleRow chains, 2 rotating banks) ----
            ybufs = [ybp.tile([F + 1, HE], bf16, name=f"yb{q}") for q in range(2)]

            def p1_block(b, g):
                yps = ps_y.tile([F + 1, EBLK], f32, tag="yps")
                for i in range(G):
                    pr = g * G + i
                    nc.tensor.matmul(
                        yps[:], lhsT=xs_sb[:, pr, :, 0:F + 1],
                        rhs=h_tiles[pr][:, :, b * EBLK:(b + 1) * EBLK],
                        start=(i == 0), stop=(i == G - 1),
                        perf_mode=DR,
                    )
                return yps

            def kick_half(q):
                b_in = dramp.tile([F + 1, HE], bf16, name=f"bi{q}")
                b_out = dramp.tile(
                    [F + 1, HE], bf16, name=f"bo{q}", addr_space="Shared"
                )
                nc.scalar.dma_start(b_in[:], ybufs[q][:])
                nc.gpsimd.collective_compute(
                    "AllReduce",
                    add,
                    ins=[b_in[:].opt()],
                    outs=[b_out[:].opt()],
                    replica_groups=[list(range(NCORES))],
                )
                # y2 half prep: XBAR transpose + de/wde + scale
                y2raw = y2rp.tile([P, HE // P, F], bf16, tag="y2r")
                nc.scalar.dma_start_transpose(y2raw[:], b_out[0:F, :])
                cs = smallp.tile([P, HE // P], bf16, name=f"cs{q}")
                nc.scalar.dma_start(
                    cs[:], b_out[F, :].rearrange("(o p) -> p o", p=P)
                )
                de_t = smallp.tile([P, HE // P], f32, name=f"de{q}")
                nc.vector.reciprocal(out=de_t[:], in_=cs[:])
                wde = smallp.tile([P, HE // P], f32, name=f"wde{q}")
                nc.vector.tensor_tensor(
                    out=wde[:], in0=de_t[:],
                    in1=wstr_sb[:, q * (HE // P):(q + 1) * (HE // P)], op=mult,
                )
                for c in range(HE // P):
                    o = q * (HE // P) + c
                    nc.scalar.activation(
                        out=y2_sb[:, o // 2, o % 2, 0:F], in_=y2raw[:, c, :],
                        func=Copy, scale=wde[:, c:c + 1],
                    )

            # phase A: group 0 over blocks 0..7 (prep of pairs 4..7 interleaves)
            for b in range(8):
                if b % 2 == 0:
                    prep_pair(G + b // 2)
                yps = p1_block(b, 0)
                nc.vector.tensor_copy(
                    out=ybufs[0][:, b * EBLK:(b + 1) * EBLK], in_=yps[:]
                )
            # phase B: group 1 over blocks 0..7 -> half 0 final, collective flies
            for b in range(8):
                yps = p1_block(b, 1)
                dst = ybufs[0][:, b * EBLK:(b + 1) * EBLK]
                nc.vector.tensor_tensor(out=dst, in0=dst, in1=yps[:], op=add)
            kick_half(0)
            # ht chunk loads go on the gpsimd/SWDGE queue so the sync and
            # scalar HWDGE queues stay free for collective bounce buffers,
            # y2 transposes and the output stores. Chunk c reuses hp slot c,
            # whose last reader is phase C (pairs 0-3) / phase D (pairs 4-7).
            chunks = []

            def load_chunks(lo, hi):
                for c in range(lo, hi):
                    ck = hp.tile([P, 4, 2, NL], f8, tag="h")
                    nc.gpsimd.dma_start(
                        ck[:],
                        ht[c * 4:(c + 1) * 4, :, :, :].rearrange(
                            "u p j n -> p u j n"
                        ),
                    )
                    chunks.append(ck)

            # phase C/D: blocks 8..15 while half-0 AllReduce is in flight
            for b in range(8, NB_BLK):
                yps = p1_block(b, 0)
                nc.vector.tensor_copy(
                    out=ybufs[1][:, (b - 8) * EBLK:(b - 7) * EBLK], in_=yps[:]
                )
            load_chunks(0, 4)
            for b in range(8, NB_BLK):
                yps = p1_block(b, 1)
                dst = ybufs[1][:, (b - 8) * EBLK:(b - 7) * EBLK]
                nc.vector.tensor_tensor(out=dst, in0=dst, in1=yps[:], op=add)
            kick_half(1)
            load_chunks(4, 8)

            # ---- pass 2: out^T accumulation via DoubleRow over e-pairs ----
            # Split per collective half: u=0..15 only needs y2 of half 0,
            # so it runs while AllReduce half 1 is still in flight.
            o_tiles = [ps_o.tile([F, NBLK], f32, name=f"o{j}") for j in range(NB)]
            for u in range(EPAIR):
                htt = chunks[u // 4]
                for j in range(NB):
                    nc.tensor.matmul(
                        o_tiles[j][:], lhsT=y2_sb[:, u, :, 0:F],
                        rhs=htt[:, u % 4, :, j * NBLK:(j + 1) * NBLK],
                        start=(u == 0), stop=(u == EPAIR - 1),
                        perf_mode=DR,
                    )

            # ---- finish: transpose back, dv/1024 scale, bias, store ----
            nc.vector.tensor_scalar_mul(dvf[:], dv_all[:], 1.0 / 1024.0)
            for j in range(NB):
                s1 = finp.tile([F, NBLK], f32, tag="s1")
                nc.vector.tensor_copy(out=s1[:], in_=o_tiles[j][:])
                osb = finp.tile([P, NBLK // P, F], f32, tag="osb")
                for c in range(NBLK // P):
                    tp = ps_y.tile([P, F], f32, tag="yps")
                    nc.tensor.transpose(
                        tp[:], s1[:, c * P:(c + 1) * P], ident[:]
                    )
                    nc.scalar.activation(
                        out=osb[:, c, :], in_=tp[:], func=Copy,
                        scale=dvf[:, j * (NBLK // P) + c:j * (NBLK // P) + c + 1],
                    )
                nc.vector.tensor_tensor(
                    out=osb[:], in0=osb[:], in1=bias_sb[:], op=add
                )
                nc.scalar.dma_start(
                    out[j * NBLK:(j + 1) * NBLK, :].rearrange(
                        "(c p) f -> p c f", p=P
                    ),
                    osb[:],
                )

    nc.finalize()
    return nc


def _get_program():
    if "nc" not in _prog_cache:
        _prog_cache["nc"] = _build_program()
    return _prog_cache["nc"]


def make_in_maps(x, H, W, weight, bias):
    x = np.asarray(x, dtype=np.float32)
    H = np.asarray(H, dtype=np.float32)
    W = np.asarray(W, dtype=np.float32)
    weight = np.asarray(weight, dtype=np.float32)
    bias = np.asarray(bias, dtype=np.float32)

    H_f8 = H.astype(ml_dtypes.float8_e4m3)
    wstr = np.ascontiguousarray((16.0 * W).reshape(ET, P).T.astype(np.float32))
    biasb = np.ascontiguousarray(
        np.tile(bias[None, None, :], (P, NB, 1)).astype(np.float32)
    )
    wmat = np.ascontiguousarray(64.0 * weight)

    in_maps = []
    for c in range(NCORES):
        hs = H_f8[c * NL:(c + 1) * NL, :]
        # natural pairs: [NPAIR, P, 2, E], (s, p, j) -> row 256 s + 128 j + p
        hpair = np.ascontiguousarray(
            hs.reshape(NPAIR, 2, P, E).transpose(0, 2, 1, 3)
        )
        # transposed pairs: [EPAIR, P, 2, NL], (u, p, j) -> col 256 u + 128 j + p
        htpair = np.ascontiguousarray(
            hs.T.reshape(EPAIR, 2, P, NL).transpose(0, 2, 1, 3)
        )
        in_maps.append({
            "h": hpair,
            "ht": htpair,
            "xt": np.ascontiguousarray(x[c * NL:(c + 1) * NL, :].T),
            "wmat": wmat,
            "wstr": wstr,
            "biasb": biasb,
        })
    return in_maps


def run(x, H, W, weight, bias, trace=False, **kw):
    from concourse.bass_utils import run_bass_kernel_spmd

    nc = _get_program()
    in_maps = make_in_maps(x, H, W, weight, bias)
    res = run_bass_kernel_spmd(nc, in_maps, list(range(NCORES)), trace=trace, **kw)
    out = np.concatenate(
        [res.results[c]["out"] for c in range(NCORES)], axis=0
    ).astype(np.float32)
    return out, res


def kernel(x, H, W, weight, bias):
    out, _ = run(x, H, W, weight, bias, trace=False)
    return out



# revision 15
# speedup vs baseline: 1.0489x; 1.0363x over previous
"""HGNN conv kernel for Trainium2, 8 NeuronCores.

out = dv ⊙ (H @ (W·de ⊙ (H^T @ (dv ⊙ (x@weight))))) + bias
  dv = rowsum(H)^-1/2  [N], de = colsum(H)^-1  [E]
  N=16384, E=8192, F=64.

Sharding: H/x row-sharded over N across 8 cores (2048 rows each).
Host preps per-core fp8(e4m3) H shards in both layouts, paired for the
tensor engine's DoubleRow mode (two 128-row chunks interleaved at the
j level) — pure layout/precision transforms; all FLOPs run on device.
fp8 is numerically safe here: the output is dominated by a coherent DC
path through the all-positive H (validated: rel err ~1.4e-3 ≪ 2e-2).
All matmul operand pairs share one dtype (mixed-dtype matmul and
tensor_tensor_reduce are hardware faults on this part - found the hard
way). Power-of-2 scales keep the fp8 operands in normal range: weight
is pre-scaled x64 (so xs=dv*xw*64), W x16 (so y2 is stored x1024), and
the final per-row scale is dv/1024.

Device per core:
  prep: xw for all 16 n-tiles into one PSUM strip. Per n-tile rowsum
        split across DVE (front half) + ACT accum (back half), summed.
  pass1: two 4-pair groups; per 512-col e-block one PSUM bank takes the
        full DoubleRow chain (2 banks ping-pong; DVE copy/add drains)
        -> ybuf bf16; half-wise AllReduce (bf16) kicks when final.
  y2:   XBAR dma-transpose of the reduced half -> [128e, 32, 64]; de
        from the ones-column strip; ACT scales by 16*W*de -> y2 e4m3.
  pass2: stream HT pair-supertiles [128, 2, 2048]; 4 persistent PSUM
        banks accumulate out^T via DoubleRow over 32 pairs; finish via
        PE transpose + ACT dv/1024 scale + DVE bias add.

v3 (from trace analysis of 316us v1 and 312us v2 runs):
(1) rowsum prep on three balanced lanes per tile: DVE direct fp8
    reduce [0:1536), ACT copy+accum [1536:4096), gpsimd fp8->f16
    fold [4096:8192) + DVE f16 reduce. (v2's DVE-side fold overloaded
    DVE to 105% busy and pushed the AR0 trigger later, 164us.)
(2) ht is repacked chunk-contiguous on host ([8, P, 4, 2, NL]) so
    each 2.1MB chunk load is one straight-run DMA on the idle Sync
    HWDGE queue - v2 put rearranged chunks on SWDGE where descriptor
    gen cost ~7.5us/chunk of GpSimd time and, queued ahead of the
    second collective trigger, delayed it to 226us.
(3) collective kicks are b_in + trigger only; the y2 prep (transpose,
    de, scales) is issued after all bulk-load DMAs so queue FIFO
    order can't block chunk loads or the AR1 trigger behind a
    transpose that waits on AR0.
(4) pass 2 u=0..15 only depends on y2 half 0, so it overlaps AR1.
"""

import numpy as np
import ml_dtypes

N, E, F = 16384, 8192, 64
NCORES = 8
NL = N // NCORES          # 2048 rows per core
P = 128
NT = NL // P              # 16 n-tiles per core
NPAIR = NT // 2           # 8 natural pair-supertiles
ET = E // P               # 64 e-tiles
EPAIR = ET // 2           # 32 transposed pair-supertiles
EBLK = 512
NB_BLK = 16               # pass-1 e-blocks of 512
HE = E // 2               # collective half width
G = 4                     # pairs per pass-1 psum chain group
NBLK = 512
NB = NL // NBLK           # 4 pass-2 output blocks
XPAD = 80                 # padded xs/y2 row length (16-aligned)

_prog_cache = {}


def _build_program():
    import concourse.bass as bass
    import concourse.mybir as mybir
    import concourse.tile as tile
    from concourse import bacc
    from concourse.masks import make_identity

    f32 = mybir.dt.float32
    bf16 = mybir.dt.bfloat16
    f16 = mybir.dt.float16
    f8 = mybir.dt.float8e4
    DR = mybir.MatmulPerfMode.DoubleRow
    Copy = mybir.ActivationFunctionType.Copy
    add = mybir.AluOpType.add
    mult = mybir.AluOpType.mult
    X = mybir.AxisListType.X

    nc = bacc.Bacc(
        "TRN2", target_bir_lowering=False, debug=False, num_devices=NCORES
    )
    h = nc.declare_dram_parameter("h", [NPAIR, P, 2, E], f8, isOutput=False)
    ht = nc.declare_dram_parameter(
        "ht", [EPAIR // 4, P, 4, 2, NL], f8, isOutput=False
    )
    xt = nc.declare_dram_parameter("xt", [F, NL], f32, isOutput=False)
    wmat = nc.declare_dram_parameter("wmat", [F, F], f32, isOutput=False)
    wstr = nc.declare_dram_parameter("wstr", [P, ET], f32, isOutput=False)
    biasb = nc.declare_dram_parameter("biasb", [P, NB, F], f32, isOutput=False)
    out = nc.declare_dram_parameter("out", [NL, F], f32, isOutput=True)

    with tile.TileContext(nc) as tc:
        with (
            tc.tile_pool(name="hp", bufs=NPAIR) as hp,           # resident H pairs
            tc.tile_pool(name="smallp", bufs=1) as smallp,       # persistent small
            tc.tile_pool(name="rp", bufs=4) as rp,               # rowsum temps
            tc.tile_pool(name="gfp", bufs=2) as gfp,             # gpsimd fold out
            tc.tile_pool(name="ybp", bufs=1) as ybp,             # ybuf halves
            tc.tile_pool(name="y2rp", bufs=1) as y2rp,           # y2 raw staging
            tc.tile_pool(name="finp", bufs=2) as finp,           # out staging
            tc.tile_pool(name="ps_xw", bufs=1, space="PSUM") as ps_xw,
            tc.tile_pool(name="ps_y", bufs=2, space="PSUM") as ps_y,
            tc.tile_pool(name="ps_o", bufs=1, space="PSUM") as ps_o,
            tc.tile_pool(name="dramp", bufs=1, space="DRAM") as dramp,
        ):
            # ---- persistent small tensors ----
            xt_sb = smallp.tile([F, NL], f32, tag="xt")
            nc.sync.dma_start(xt_sb[:], xt[:, :])
            wmat_sb = smallp.tile([F, F], f32, tag="wmat")
            nc.sync.dma_start(wmat_sb[:], wmat[:, :])
            wstr_sb = smallp.tile([P, ET], f32, tag="wstr")
            nc.sync.dma_start(wstr_sb[:], wstr[:, :])
            bias_sb = smallp.tile([P, NB, F], f32, tag="bias")
            nc.sync.dma_start(bias_sb[:], biasb[:, :, :])
            dv_all = smallp.tile([P, NT], f32, tag="dv")
            dvf = smallp.tile([P, NT], f32, tag="dvf")
            xs_sb = smallp.tile([P, NPAIR, 2, XPAD], f8, tag="xs")
            nc.gpsimd.memset(xs_sb[:, :, :, F:F + 1], 1.0)
            y2_sb = smallp.tile([P, EPAIR, 2, XPAD], f8, tag="y2")
            dumA = smallp.tile([P, 2560], f8, tag="dumA")  # ACT rowsum sink
            ident = smallp.tile([F, F], f32, tag="ident")
            make_identity(nc, ident)

            # ---- xw = x @ weight (x64) for all n-tiles, kept in PSUM ----
            xw_all = ps_xw.tile([P, NT * F], f32, tag="xw")
            for t in range(NT):
                nc.tensor.matmul(
                    xw_all[:, t * F:(t + 1) * F],
                    lhsT=xt_sb[:, t * P:(t + 1) * P], rhs=wmat_sb[:],
                    start=True, stop=True,
                )

            h_tiles = []

            def prep_pair(s):
                h_s = hp.tile([P, 2, E], f8, tag="h")
                eng = nc.sync if s % 2 == 0 else nc.scalar
                eng.dma_start(h_s[:], h[s, :, :, :])
                h_tiles.append(h_s)
                for j in range(2):
                    t = 2 * s + j
                    # rowsum 3-lane split:
                    #  DVE direct fp8 reduce [0:1536)
                    #  ACT copy+accum [1536:4096)
                    #  gpsimd fp8-fold [4096:8192) -> f16 [P,2048] -> DVE
                    gfo = gfp.tile([P, 2048], f16, tag="gf")
                    nc.gpsimd.tensor_tensor(
                        out=gfo[:], in0=h_s[:, j, 4096:6144],
                        in1=h_s[:, j, 6144:8192], op=add,
                    )
                    r0 = rp.tile([P, 1], f32, tag="r0")
                    nc.vector.tensor_reduce(
                        out=r0[:], in_=h_s[:, j, 0:1536], axis=X, op=add
                    )
                    r1 = rp.tile([P, 1], f32, tag="r1")
                    nc.scalar.activation(
                        out=dumA[:], in_=h_s[:, j, 1536:4096], func=Copy,
                        accum_out=r1[:],
                    )
                    r2 = rp.tile([P, 1], f32, tag="r2")
                    nc.vector.tensor_reduce(
                        out=r2[:], in_=gfo[:], axis=X, op=add
                    )
                    rinv = rp.tile([P, 1], f32, tag="rinv")
                    nc.vector.tensor_tensor(out=rinv[:], in0=r0[:], in1=r1[:], op=add)
                    nc.vector.tensor_tensor(out=rinv[:], in0=rinv[:], in1=r2[:], op=add)
                    nc.vector.reciprocal(out=rinv[:], in_=rinv[:])
                    nc.scalar.sqrt(out=dv_all[:, t:t + 1], in_=rinv[:])
                    nc.scalar.activation(
                        out=xs_sb[:, s, j, 0:F], in_=xw_all[:, t * F:(t + 1) * F],
                        func=Copy, scale=dv_all[:, t:t + 1],
                    )

            for s in range(G):
                prep_pair(s)

            # ---- pass 1 (DoubleRow chains, 2 rotating banks) ----
            ybufs = [ybp.tile([F + 1, HE], bf16, name=f"yb{q}") for q in range(2)]

            def p1_block(b, g):
                yps = ps_y.tile([F + 1, EBLK], f32, tag="yps")
                for i in range(G):
                    pr = g * G + i
                    nc.tensor.matmul(
                        yps[:], lhsT=xs_sb[:, pr, :, 0:F + 1],
                        rhs=h_tiles[pr][:, :, b * EBLK:(b + 1) * EBLK],
                        start=(i == 0), stop=(i == G - 1),
                        perf_mode=DR,
                    )
                return yps

            b_outs = []

            def kick_half(q):
                b_in = dramp.tile([F + 1, HE], bf16, name=f"bi{q}")
                b_out = dramp.tile(
                    [F + 1, HE], bf16, name=f"bo{q}", addr_space="Shared"
                )
                nc.scalar.dma_start(b_in[:], ybufs[q][:])
                nc.gpsimd.collective_compute(
                    "AllReduce",
                    add,
                    ins=[b_in[:].opt()],
                    outs=[b_out[:].opt()],
                    replica_groups=[list(range(NCORES))],
                )
                b_outs.append(b_out)

            def y2_prep(q):
                # y2 half prep: XBAR transpose + de/wde + scale
                b_out = b_outs[q]
                y2raw = y2rp.tile([P, HE // P, F], bf16, tag="y2r")
                nc.scalar.dma_start_transpose(y2raw[:], b_out[0:F, :])
                cs = smallp.tile([P, HE // P], bf16, name=f"cs{q}")
                nc.scalar.dma_start(
                    cs[:], b_out[F, :].rearrange("(o p) -> p o", p=P)
                )
                de_t = smallp.tile([P, HE // P], f32, name=f"de{q}")
                nc.vector.reciprocal(out=de_t[:], in_=cs[:])
                wde = smallp.tile([P, HE // P], f32, name=f"wde{q}")
                nc.vector.tensor_tensor(
                    out=wde[:], in0=de_t[:],
                    in1=wstr_sb[:, q * (HE // P):(q + 1) * (HE // P)], op=mult,
                )
                for c in range(HE // P):
                    o = q * (HE // P) + c
                    nc.scalar.activation(
                        out=y2_sb[:, o // 2, o % 2, 0:F], in_=y2raw[:, c, :],
                        func=Copy, scale=wde[:, c:c + 1],
                    )

            # phase A: group 0 over blocks 0..7 (prep of pairs 4..7 interleaves)
            for b in range(8):
                if b % 2 == 0:
                    prep_pair(G + b // 2)
                yps = p1_block(b, 0)
                nc.vector.tensor_copy(
                    out=ybufs[0][:, b * EBLK:(b + 1) * EBLK], in_=yps[:]
                )
            # phase B: group 1 over blocks 0..7 -> half 0 final, collective flies
            for b in range(8):
                yps = p1_block(b, 1)
                dst = ybufs[0][:, b * EBLK:(b + 1) * EBLK]
                nc.vector.tensor_tensor(out=dst, in0=dst, in1=yps[:], op=add)
            kick_half(0)
            # ht chunks are host-packed contiguous ([8, P, 4, 2, NL]) so each
            # load is one straight-run HWDGE DMA on the idle Sync queue.
            # Chunk c reuses hp slot c, whose last reader is phase C
            # (pairs 0-3) / phase D (pairs 4-7).
            chunks = []

            def load_chunks(lo, hi):
                for c in range(lo, hi):
                    ck = hp.tile([P, 4, 2, NL], f8, tag="h")
                    nc.sync.dma_start(ck[:], ht[c, :, :, :, :])
                    chunks.append(ck)

            # phase C/D: blocks 8..15 while half-0 AllReduce is in flight
            for b in range(8, NB_BLK):
                yps = p1_block(b, 0)
                nc.vector.tensor_copy(
                    out=ybufs[1][:, (b - 8) * EBLK:(b - 7) * EBLK], in_=yps[:]
                )
            load_chunks(0, 4)
            for b in range(8, NB_BLK):
                yps = p1_block(b, 1)
                dst = ybufs[1][:, (b - 8) * EBLK:(b - 7) * EBLK]
                nc.vector.tensor_tensor(out=dst, in0=dst, in1=yps[:], op=add)
            kick_half(1)
            load_chunks(4, 8)
            y2_prep(0)

            # ---- pass 2: out^T accumulation via DoubleRow over e-pairs ----
            # u=0..15 only needs y2 half 0 + chunks 0-3, so it overlaps the
            # half-1 AllReduce still in flight.
            o_tiles = [ps_o.tile([F, NBLK], f32, name=f"o{j}") for j in range(NB)]
            for u in range(EPAIR // 2):
                htt = chunks[u // 4]
                for j in range(NB):
                    nc.tensor.matmul(
                        o_tiles[j][:], lhsT=y2_sb[:, u, :, 0:F],
                        rhs=htt[:, u % 4, :, j * NBLK:(j + 1) * NBLK],
                        start=(u == 0), stop=False,
                        perf_mode=DR,
                    )
            y2_prep(1)
            for u in range(EPAIR // 2, EPAIR):
                htt = chunks[u // 4]
                for j in range(NB):
                    nc.tensor.matmul(
                        o_tiles[j][:], lhsT=y2_sb[:, u, :, 0:F],
                        rhs=htt[:, u % 4, :, j * NBLK:(j + 1) * NBLK],
                        start=False, stop=(u == EPAIR - 1),
                        perf_mode=DR,
                    )

            # ---- finish: transpose back, dv/1024 scale, bias, store ----
            nc.vector.tensor_scalar_mul(dvf[:], dv_all[:], 1.0 / 1024.0)
            for j in range(NB):
                s1 = finp.tile([F, NBLK], f32, tag="s1")
                nc.vector.tensor_copy(out=s1[:], in_=o_tiles[j][:])
                osb = finp.tile([P, NBLK // P, F], f32, tag="osb")
                for c in range(NBLK // P):
                    tp = ps_y.tile([P, F], f32, tag="yps")
                    nc.tensor.transpose(
                        tp[:], s1[:, c * P:(c + 1) * P], ident[:]
                    )
                    nc.scalar.activation(
                        out=osb[:, c, :], in_=tp[:], func=Copy,
                        scale=dvf[:, j * (NBLK // P) + c:j * (NBLK // P) + c + 1],
                    )
                nc.vector.tensor_tensor(
                    out=osb[:], in0=osb[:], in1=bias_sb[:], op=add
                )
                nc.scalar.dma_start(
                    out[j * NBLK:(j + 1) * NBLK, :].rearrange(
                        "(c p) f -> p c f", p=P
                    ),
                    osb[:],
                )

    nc.finalize()
    return nc


def _get_program():
    if "nc" not in _prog_cache:
        _prog_cache["nc"] = _build_program()
    return _prog_cache["nc"]


def make_in_maps(x, H, W, weight, bias):
    x = np.asarray(x, dtype=np.float32)
    H = np.asarray(H, dtype=np.float32)
    W = np.asarray(W, dtype=np.float32)
    weight = np.asarray(weight, dtype=np.float32)
    bias = np.asarray(bias, dtype=np.float32)

    H_f8 = H.astype(ml_dtypes.float8_e4m3)
    wstr = np.ascontiguousarray((16.0 * W).reshape(ET, P).T.astype(np.float32))
    biasb = np.ascontiguousarray(
        np.tile(bias[None, None, :], (P, NB, 1)).astype(np.float32)
    )
    wmat = np.ascontiguousarray(64.0 * weight)

    in_maps = []
    for c in range(NCORES):
        hs = H_f8[c * NL:(c + 1) * NL, :]
        # natural pairs: [NPAIR, P, 2, E], (s, p, j) -> row 256 s + 128 j + p
        hpair = np.ascontiguousarray(
            hs.reshape(NPAIR, 2, P, E).transpose(0, 2, 1, 3)
        )
        # transposed pairs, chunk-contiguous: [8, P, 4, 2, NL],
        # (c, p, u4, j) -> col 256 (4c + u4) + 128 j + p
        htpair = np.ascontiguousarray(
            hs.T.reshape(8, 4, 2, P, NL).transpose(0, 3, 1, 2, 4)
        )
        in_maps.append({
            "h": hpair,
            "ht": htpair,
            "xt": np.ascontiguousarray(x[c * NL:(c + 1) * NL, :].T),
            "wmat": wmat,
            "wstr": wstr,
            "biasb": biasb,
        })
    return in_maps


def run(x, H, W, weight, bias, trace=False, **kw):
    from concourse.bass_utils import run_bass_kernel_spmd

    nc = _get_program()
    in_maps = make_in_maps(x, H, W, weight, bias)
    res = run_bass_kernel_spmd(nc, in_maps, list(range(NCORES)), trace=trace, **kw)
    out = np.concatenate(
        [res.results[c]["out"] for c in range(NCORES)], axis=0
    ).astype(np.float32)
    return out, res


def kernel(x, H, W, weight, bias):
    out, _ = run(x, H, W, weight, bias, trace=False)
    return out



# revision 19
# speedup vs baseline: 1.1455x; 1.0921x over previous
"""HGNN conv kernel for Trainium2, 8 NeuronCores.

out = dv ⊙ (H @ (W·de ⊙ (H^T @ (dv ⊙ (x@weight))))) + bias
  dv = rowsum(H)^-1/2  [N], de = colsum(H)^-1  [E]
  N=16384, E=8192, F=64.

Sharding: H/x row-sharded over N across 8 cores (2048 rows each).
Host preps per-core fp8(e4m3) H shards in both layouts, paired for the
tensor engine's DoubleRow mode (two 128-row chunks interleaved at the
j level) — pure layout/precision transforms; all FLOPs run on device.
fp8 is numerically safe here: the output is dominated by a coherent DC
path through the all-positive H (validated: rel err ~1.4e-3 ≪ 2e-2).
All matmul operand pairs share one dtype (mixed-dtype matmul and
tensor_tensor_reduce are hardware faults on this part - found the hard
way). Power-of-2 scales keep the fp8 operands in normal range: weight
is pre-scaled x64 (so xs=dv*xw*64), W x16 (so y2 is stored x1024), and
the final per-row scale is dv/1024.

Device per core:
  prep: xw for all 16 n-tiles into one PSUM strip. Per n-tile rowsum
        split across DVE (front half) + ACT accum (back half), summed.
  pass1: two 4-pair groups; per 512-col e-block one PSUM bank takes the
        full DoubleRow chain (2 banks ping-pong; DVE copy/add drains)
        -> ybuf bf16; half-wise AllReduce (bf16) kicks when final.
  y2:   XBAR dma-transpose of the reduced half -> [128e, 32, 64]; de
        from the ones-column strip; ACT scales by 16*W*de -> y2 e4m3.
  pass2: stream HT pair-supertiles [128, 2, 2048]; 4 persistent PSUM
        banks accumulate out^T via DoubleRow over 32 pairs; finish via
        PE transpose + ACT dv/1024 scale + DVE bias add.

v3 (from trace analysis of 316us v1 and 312us v2 runs):
(1) rowsum prep on three balanced lanes per tile: DVE direct fp8
    reduce [0:1536), ACT copy+accum [1536:4096), gpsimd fp8->f16
    fold [4096:8192) + DVE f16 reduce. (v2's DVE-side fold overloaded
    DVE to 105% busy and pushed the AR0 trigger later, 164us.)
(2) ht is repacked chunk-contiguous on host ([8, P, 4, 2, NL]) so
    each 2.1MB chunk load is one straight-run DMA on the idle Sync
    HWDGE queue - v2 put rearranged chunks on SWDGE where descriptor
    gen cost ~7.5us/chunk of GpSimd time and, queued ahead of the
    second collective trigger, delayed it to 226us.
(3) collective kicks are b_in + trigger only; the y2 prep (transpose,
    de, scales) is issued after all bulk-load DMAs so queue FIFO
    order can't block chunk loads or the AR1 trigger behind a
    transpose that waits on AR0.
(4) pass 2 u=0..15 only depends on y2 half 0, so it overlaps AR1.
"""

import numpy as np
import ml_dtypes

N, E, F = 16384, 8192, 64
NCORES = 8
NL = N // NCORES          # 2048 rows per core
P = 128
NT = NL // P              # 16 n-tiles per core
NPAIR = NT // 2           # 8 natural pair-supertiles
ET = E // P               # 64 e-tiles
EPAIR = ET // 2           # 32 transposed pair-supertiles
EBLK = 512
NB_BLK = 16               # pass-1 e-blocks of 512
HE = E // 2               # collective half width
G = 4                     # pairs per pass-1 psum chain group
NBLK = 512
NB = NL // NBLK           # 4 pass-2 output blocks
XPAD = 80                 # padded xs/y2 row length (16-aligned)

_prog_cache = {}


def _build_program():
    import concourse.bass as bass
    import concourse.mybir as mybir
    import concourse.tile as tile
    from concourse import bacc
    from concourse.masks import make_identity

    f32 = mybir.dt.float32
    bf16 = mybir.dt.bfloat16
    f16 = mybir.dt.float16
    f8 = mybir.dt.float8e4
    DR = mybir.MatmulPerfMode.DoubleRow
    Copy = mybir.ActivationFunctionType.Copy
    add = mybir.AluOpType.add
    mult = mybir.AluOpType.mult
    X = mybir.AxisListType.X

    nc = bacc.Bacc(
        "TRN2", target_bir_lowering=False, debug=False, num_devices=NCORES
    )
    h = nc.declare_dram_parameter("h", [NPAIR, P, 2, E], f8, isOutput=False)
    ht = nc.declare_dram_parameter(
        "ht", [EPAIR // 4, P, 4, 2, NL], f8, isOutput=False
    )
    xt = nc.declare_dram_parameter("xt", [F, NL], f32, isOutput=False)
    wmat = nc.declare_dram_parameter("wmat", [F, F], f32, isOutput=False)
    wstr = nc.declare_dram_parameter("wstr", [P, ET], f32, isOutput=False)
    biasb = nc.declare_dram_parameter("biasb", [P, NB, F], f32, isOutput=False)
    out = nc.declare_dram_parameter("out", [NL, F], f32, isOutput=True)

    with tile.TileContext(nc) as tc:
        with (
            tc.tile_pool(name="hp", bufs=NPAIR) as hp,           # resident H pairs
            tc.tile_pool(name="smallp", bufs=1) as smallp,       # persistent small
            tc.tile_pool(name="rp", bufs=4) as rp,               # rowsum temps
            tc.tile_pool(name="gfp", bufs=2) as gfp,             # gpsimd fold out
            tc.tile_pool(name="ybp", bufs=1) as ybp,             # ybuf halves
            tc.tile_pool(name="y2rp", bufs=1) as y2rp,           # y2 raw staging
            tc.tile_pool(name="finp", bufs=2) as finp,           # out staging
            tc.tile_pool(name="ps_xw", bufs=1, space="PSUM") as ps_xw,
            tc.tile_pool(name="ps_y", bufs=2, space="PSUM") as ps_y,
            tc.tile_pool(name="ps_o", bufs=1, space="PSUM") as ps_o,
            tc.tile_pool(name="dramp", bufs=1, space="DRAM") as dramp,
        ):
            # ---- persistent small tensors ----
            xt_sb = smallp.tile([F, NL], f32, tag="xt")
            nc.sync.dma_start(xt_sb[:], xt[:, :])
            wmat_sb = smallp.tile([F, F], f32, tag="wmat")
            nc.sync.dma_start(wmat_sb[:], wmat[:, :])
            wstr_sb = smallp.tile([P, ET], f32, tag="wstr")
            nc.sync.dma_start(wstr_sb[:], wstr[:, :])
            bias_sb = smallp.tile([P, NB, F], f32, tag="bias")
            nc.sync.dma_start(bias_sb[:], biasb[:, :, :])
            dv_all = smallp.tile([P, NT], f32, tag="dv")
            dvf = smallp.tile([P, NT], f32, tag="dvf")
            r0_all = smallp.tile([P, NT], f32, tag="r0a")
            r1_all = smallp.tile([P, NT], f32, tag="r1a")
            r2_all = smallp.tile([P, NT], f32, tag="r2a")
            xs_sb = smallp.tile([P, NPAIR, 2, XPAD], f8, tag="xs")
            nc.gpsimd.memset(xs_sb[:, :, :, F:F + 1], 1.0)
            y2_sb = smallp.tile([P, EPAIR, 2, XPAD], f8, tag="y2")
            dumA = smallp.tile([P, 2560], f8, tag="dumA")  # ACT rowsum sink
            ident = smallp.tile([F, F], f32, tag="ident")
            make_identity(nc, ident)

            # ---- xw = x @ weight (x64) for all n-tiles, kept in PSUM ----
            xw_all = ps_xw.tile([P, NT * F], f32, tag="xw")
            for t in range(NT):
                nc.tensor.matmul(
                    xw_all[:, t * F:(t + 1) * F],
                    lhsT=xt_sb[:, t * P:(t + 1) * P], rhs=wmat_sb[:],
                    start=True, stop=True,
                )

            h_tiles = []

            def prep_pair(s):
                h_s = hp.tile([P, 2, E], f8, tag="h")
                eng = nc.sync if s % 2 == 0 else nc.scalar
                eng.dma_start(h_s[:], h[s, :, :, :])
                h_tiles.append(h_s)
                for j in range(2):
                    t = 2 * s + j
                    # rowsum 3-lane split:
                    #  DVE direct fp8 reduce [0:1536)
                    #  ACT copy+accum [1536:4096)
                    #  gpsimd fp8-fold [4096:8192) -> f16 [P,2048] -> DVE
                    gfo = gfp.tile([P, 2048], f16, tag="gf")
                    nc.gpsimd.tensor_tensor(
                        out=gfo[:], in0=h_s[:, j, 4096:6144],
                        in1=h_s[:, j, 6144:8192], op=add,
                    )
                    nc.vector.tensor_reduce(
                        out=r0_all[:, t:t + 1], in_=h_s[:, j, 0:1536],
                        axis=X, op=add,
                    )
                    nc.scalar.activation(
                        out=dumA[:], in_=h_s[:, j, 1536:4096], func=Copy,
                        accum_out=r1_all[:, t:t + 1],
                    )
                    nc.vector.tensor_reduce(
                        out=r2_all[:, t:t + 1], in_=gfo[:], axis=X, op=add
                    )

            def finalize_group(g):
                # batched dv + xs for 8 tiles: one TT/TT/recip/sqrt instead
                # of per-tile [P,1] ops (those cost ~1us fixed overhead each)
                lo, hi = g * 8, g * 8 + 8
                rs = rp.tile([P, 8], f32, tag="rs")
                nc.vector.tensor_tensor(
                    out=rs[:], in0=r0_all[:, lo:hi], in1=r1_all[:, lo:hi], op=add
                )
                nc.vector.tensor_tensor(
                    out=rs[:], in0=rs[:], in1=r2_all[:, lo:hi], op=add
                )
                nc.vector.reciprocal(out=rs[:], in_=rs[:])
                nc.scalar.sqrt(out=dv_all[:, lo:hi], in_=rs[:])
                for t in range(lo, hi):
                    nc.scalar.activation(
                        out=xs_sb[:, t // 2, t % 2, 0:F],
                        in_=xw_all[:, t * F:(t + 1) * F],
                        func=Copy, scale=dv_all[:, t:t + 1],
                    )

            for s in range(G):
                prep_pair(s)
            finalize_group(0)

            # ---- pass 1 (DoubleRow chains, 2 rotating banks) ----
            ybufs = [ybp.tile([F + 1, HE], bf16, name=f"yb{q}") for q in range(2)]

            def p1_block(b, g):
                yps = ps_y.tile([F + 1, EBLK], f32, tag="yps")
                for i in range(G):
                    pr = g * G + i
                    nc.tensor.matmul(
                        yps[:], lhsT=xs_sb[:, pr, :, 0:F + 1],
                        rhs=h_tiles[pr][:, :, b * EBLK:(b + 1) * EBLK],
                        start=(i == 0), stop=(i == G - 1),
                        perf_mode=DR,
                    )
                return yps

            b_outs = []

            def kick_half(q):
                b_in = dramp.tile([F + 1, HE], bf16, name=f"bi{q}")
                b_out = dramp.tile(
                    [F + 1, HE], bf16, name=f"bo{q}", addr_space="Shared"
                )
                nc.scalar.dma_start(b_in[:], ybufs[q][:])
                nc.gpsimd.collective_compute(
                    "AllReduce",
                    add,
                    ins=[b_in[:].opt()],
                    outs=[b_out[:].opt()],
                    replica_groups=[list(range(NCORES))],
                )
                b_outs.append(b_out)

            def y2_prep(q):
                # y2 half prep: XBAR transpose + de/wde + scale
                b_out = b_outs[q]
                y2raw = y2rp.tile([P, HE // P, F], bf16, tag="y2r")
                nc.scalar.dma_start_transpose(y2raw[:], b_out[0:F, :])
                cs = smallp.tile([P, HE // P], bf16, name=f"cs{q}")
                nc.scalar.dma_start(
                    cs[:], b_out[F, :].rearrange("(o p) -> p o", p=P)
                )
                de_t = smallp.tile([P, HE // P], f32, name=f"de{q}")
                nc.vector.reciprocal(out=de_t[:], in_=cs[:])
                wde = smallp.tile([P, HE // P], f32, name=f"wde{q}")
                nc.vector.tensor_tensor(
                    out=wde[:], in0=de_t[:],
                    in1=wstr_sb[:, q * (HE // P):(q + 1) * (HE // P)], op=mult,
                )
                for c in range(HE // P):
                    o = q * (HE // P) + c
                    nc.scalar.activation(
                        out=y2_sb[:, o // 2, o % 2, 0:F], in_=y2raw[:, c, :],
                        func=Copy, scale=wde[:, c:c + 1],
                    )

            # phase A: group 0 over blocks 0..7 (prep of pairs 4..7 interleaves)
            for b in range(8):
                if b % 2 == 0:
                    prep_pair(G + b // 2)
                yps = p1_block(b, 0)
                nc.vector.tensor_copy(
                    out=ybufs[0][:, b * EBLK:(b + 1) * EBLK], in_=yps[:]
                )
            finalize_group(1)
            # phase B: group 1 over blocks 0..7 -> half 0 final, collective flies
            for b in range(8):
                yps = p1_block(b, 1)
                dst = ybufs[0][:, b * EBLK:(b + 1) * EBLK]
                nc.vector.tensor_tensor(out=dst, in0=dst, in1=yps[:], op=add)
            kick_half(0)
            # ht chunks are host-packed contiguous ([8, P, 4, 2, NL]) so each
            # load is one straight-run HWDGE DMA on the idle Sync queue.
            # Chunk c reuses hp slot c, whose last reader is phase C
            # (pairs 0-3) / phase D (pairs 4-7).
            chunks = []

            def load_chunks(lo, hi):
                for c in range(lo, hi):
                    ck = hp.tile([P, 4, 2, NL], f8, tag="h")
                    nc.sync.dma_start(ck[:], ht[c, :, :, :, :])
                    chunks.append(ck)

            # phase C/D: blocks 8..15 while half-0 AllReduce is in flight
            for b in range(8, NB_BLK):
                yps = p1_block(b, 0)
                nc.vector.tensor_copy(
                    out=ybufs[1][:, (b - 8) * EBLK:(b - 7) * EBLK], in_=yps[:]
                )
            load_chunks(0, 4)
            for b in range(8, NB_BLK):
                yps = p1_block(b, 1)
                dst = ybufs[1][:, (b - 8) * EBLK:(b - 7) * EBLK]
                nc.vector.tensor_tensor(out=dst, in0=dst, in1=yps[:], op=add)
            kick_half(1)
            load_chunks(4, 8)
            y2_prep(0)

            # ---- pass 2: out^T accumulation via DoubleRow over e-pairs ----
            # u=0..15 only needs y2 half 0 + chunks 0-3, so it overlaps the
            # half-1 AllReduce still in flight.
            o_tiles = [ps_o.tile([F, NBLK], f32, name=f"o{j}") for j in range(NB)]
            for u in range(EPAIR // 2):
                htt = chunks[u // 4]
                for j in range(NB):
                    nc.tensor.matmul(
                        o_tiles[j][:], lhsT=y2_sb[:, u, :, 0:F],
                        rhs=htt[:, u % 4, :, j * NBLK:(j + 1) * NBLK],
                        start=(u == 0), stop=False,
                        perf_mode=DR,
                    )
            y2_prep(1)
            for u in range(EPAIR // 2, EPAIR):
                htt = chunks[u // 4]
                for j in range(NB):
                    nc.tensor.matmul(
                        o_tiles[j][:], lhsT=y2_sb[:, u, :, 0:F],
                        rhs=htt[:, u % 4, :, j * NBLK:(j + 1) * NBLK],
                        start=False, stop=(u == EPAIR - 1),
                        perf_mode=DR,
                    )

            # ---- finish: transpose back, dv/1024 scale, bias, store ----
            nc.vector.tensor_scalar_mul(dvf[:], dv_all[:], 1.0 / 1024.0)
            for j in range(NB):
                s1 = finp.tile([F, NBLK], f32, tag="s1")
                nc.vector.tensor_copy(out=s1[:], in_=o_tiles[j][:])
                osb = finp.tile([P, NBLK // P, F], f32, tag="osb")
                for c in range(NBLK // P):
                    tp = ps_y.tile([P, F], f32, tag="yps")
                    nc.tensor.transpose(
                        tp[:], s1[:, c * P:(c + 1) * P], ident[:]
                    )
                    nc.scalar.activation(
                        out=osb[:, c, :], in_=tp[:], func=Copy,
                        scale=dvf[:, j * (NBLK // P) + c:j * (NBLK // P) + c + 1],
                    )
                nc.vector.tensor_tensor(
                    out=osb[:], in0=osb[:], in1=bias_sb[:], op=add
                )
                nc.scalar.dma_start(
                    out[j * NBLK:(j + 1) * NBLK, :].rearrange(
                        "(c p) f -> p c f", p=P
                    ),
                    osb[:],
                )

    nc.finalize()
    return nc


def _get_program():
    if "nc" not in _prog_cache:
        _prog_cache["nc"] = _build_program()
    return _prog_cache["nc"]


def make_in_maps(x, H, W, weight, bias):
    x = np.asarray(x, dtype=np.float32)
    H = np.asarray(H, dtype=np.float32)
    W = np.asarray(W, dtype=np.float32)
    weight = np.asarray(weight, dtype=np.float32)
    bias = np.asarray(bias, dtype=np.float32)

    H_f8 = H.astype(ml_dtypes.float8_e4m3)
    wstr = np.ascontiguousarray((16.0 * W).reshape(ET, P).T.astype(np.float32))
    biasb = np.ascontiguousarray(
        np.tile(bias[None, None, :], (P, NB, 1)).astype(np.float32)
    )
    wmat = np.ascontiguousarray(64.0 * weight)

    in_maps = []
    for c in range(NCORES):
        hs = H_f8[c * NL:(c + 1) * NL, :]
        # natural pairs: [NPAIR, P, 2, E], (s, p, j) -> row 256 s + 128 j + p
        hpair = np.ascontiguousarray(
            hs.reshape(NPAIR, 2, P, E).transpose(0, 2, 1, 3)
        )
        # transposed pairs, chunk-contiguous: [8, P, 4, 2, NL],
        # (c, p, u4, j) -> col 256 (4c + u4) + 128 j + p
        htpair = np.ascontiguousarray(
            hs.T.reshape(8, 4, 2, P, NL).transpose(0, 3, 1, 2, 4)
        )
        in_maps.append({
            "h": hpair,
            "ht": htpair,
            "xt": np.ascontiguousarray(x[c * NL:(c + 1) * NL, :].T),
            "wmat": wmat,
            "wstr": wstr,
            "biasb": biasb,
        })
    return in_maps


def run(x, H, W, weight, bias, trace=False, **kw):
    from concourse.bass_utils import run_bass_kernel_spmd

    nc = _get_program()
    in_maps = make_in_maps(x, H, W, weight, bias)
    res = run_bass_kernel_spmd(nc, in_maps, list(range(NCORES)), trace=trace, **kw)
    out = np.concatenate(
        [res.results[c]["out"] for c in range(NCORES)], axis=0
    ).astype(np.float32)
    return out, res


def kernel(x, H, W, weight, bias):
    out, _ = run(x, H, W, weight, bias, trace=False)
    return out



# revision 22
# speedup vs baseline: 1.1795x; 1.0297x over previous
"""HGNN conv kernel for Trainium2, 8 NeuronCores.

out = dv ⊙ (H @ (W·de ⊙ (H^T @ (dv ⊙ (x@weight))))) + bias
  dv = rowsum(H)^-1/2  [N], de = colsum(H)^-1  [E]
  N=16384, E=8192, F=64.

Sharding: H/x row-sharded over N across 8 cores (2048 rows each).
Host preps per-core fp8(e4m3) H shards in both layouts, paired for the
tensor engine's DoubleRow mode (two 128-row chunks interleaved at the
j level) — pure layout/precision transforms; all FLOPs run on device.
fp8 is numerically safe here: the output is dominated by a coherent DC
path through the all-positive H (validated: rel err ~1.4e-3 ≪ 2e-2).
All matmul operand pairs share one dtype (mixed-dtype matmul and
tensor_tensor_reduce are hardware faults on this part - found the hard
way). Power-of-2 scales keep the fp8 operands in normal range: weight
is pre-scaled x64 (so xs=dv*xw*64), W x16 (so y2 is stored x1024), and
the final per-row scale is dv/1024.

Device per core:
  prep: xw for all 16 n-tiles into one PSUM strip. Per n-tile rowsum
        split across DVE (front half) + ACT accum (back half), summed.
  pass1: two 4-pair groups; per 512-col e-block one PSUM bank takes the
        full DoubleRow chain (2 banks ping-pong; DVE copy/add drains)
        -> ybuf bf16; half-wise AllReduce (bf16) kicks when final.
  y2:   XBAR dma-transpose of the reduced half -> [128e, 32, 64]; de
        from the ones-column strip; ACT scales by 16*W*de -> y2 e4m3.
  pass2: stream HT pair-supertiles [128, 2, 2048]; 4 persistent PSUM
        banks accumulate out^T via DoubleRow over 32 pairs; finish via
        PE transpose + ACT dv/1024 scale + DVE bias add.

Measured 275,980 ns (vs 316,125 ns at session start; traced runs).
Session changes, each verified by trace:
(1) rowsum prep on three balanced lanes per tile: DVE direct fp8
    reduce [0:1536), ACT copy+accum [1536:4096), gpsimd fp8->f16
    fold [4096:8192) + DVE f16 reduce. NOTE: f16 tensor_reduce runs
    at the SAME ~0.9 elem/ns/lane as fp8 (no 16-bit 2x observed), and
    a DVE-side fold is counterproductive (overloads DVE).
(2) dv combines batched per 4-pair group into [P,8] strip ops - the
    per-tile [P,1] tensor_tensor adds cost ~975ns EACH of pure
    instruction overhead (31us of DVE on trivial adds).
(3) ht repacked chunk-contiguous on host ([8, P, 4, 2, NL]) so each
    2.1MB chunk load is one straight-run DMA on the idle Sync HWDGE
    queue. SWDGE descriptor gen for the rearranged view costs
    ~7.5us/chunk of GpSimd time and delays the AR1 trigger queued
    behind it.
(4) collective kicks are b_in + trigger only; y2 prep (transpose, de,
    scales) is issued after all bulk-load DMAs so scalar-queue FIFO
    order can't block the AR1 trigger behind a transpose waiting on
    AR0. Pass-2 u=0..15 depends only on y2 half 0, overlapping AR1.
Dead end (tested, 295,610 ns - do NOT redo): splitting the collective
into 4 quarter-AllReduces. The first AR's ~51us duration is an
inter-core skew/rendezvous constant (identical at 0.53MB and 0.27MB
payloads, bus_bw scales down to match); splitting just adds ~15us
floor per extra AR while the skew penalty stays.
Remaining bottlenecks in the 276us trace: first AR exposed ~51us
(130-181us, absorbs all-core skew; payload-independent), prep lanes
40-100us gate pass-1 so AR0 triggers at ~119us, pass-2 200-270us,
finish tail ~14us. Theoretical floor ~200us unless the skew source
is found (candidates: HBM port sharing between paired NCs during the
16.8MB h load, initial barrier exit spread).
"""

import numpy as np
import ml_dtypes

N, E, F = 16384, 8192, 64
NCORES = 8
NL = N // NCORES          # 2048 rows per core
P = 128
NT = NL // P              # 16 n-tiles per core
NPAIR = NT // 2           # 8 natural pair-supertiles
ET = E // P               # 64 e-tiles
EPAIR = ET // 2           # 32 transposed pair-supertiles
EBLK = 512
NB_BLK = 16               # pass-1 e-blocks of 512
HE = E // 2               # collective half width
G = 4                     # pairs per pass-1 psum chain group
NBLK = 512
NB = NL // NBLK           # 4 pass-2 output blocks
XPAD = 80                 # padded xs/y2 row length (16-aligned)

_prog_cache = {}


def _build_program():
    import concourse.bass as bass
    import concourse.mybir as mybir
    import concourse.tile as tile
    from concourse import bacc
    from concourse.masks import make_identity

    f32 = mybir.dt.float32
    bf16 = mybir.dt.bfloat16
    f16 = mybir.dt.float16
    f8 = mybir.dt.float8e4
    DR = mybir.MatmulPerfMode.DoubleRow
    Copy = mybir.ActivationFunctionType.Copy
    add = mybir.AluOpType.add
    mult = mybir.AluOpType.mult
    X = mybir.AxisListType.X

    nc = bacc.Bacc(
        "TRN2", target_bir_lowering=False, debug=False, num_devices=NCORES
    )
    h = nc.declare_dram_parameter("h", [NPAIR, P, 2, E], f8, isOutput=False)
    ht = nc.declare_dram_parameter(
        "ht", [EPAIR // 4, P, 4, 2, NL], f8, isOutput=False
    )
    xt = nc.declare_dram_parameter("xt", [F, NL], f32, isOutput=False)
    wmat = nc.declare_dram_parameter("wmat", [F, F], f32, isOutput=False)
    wstr = nc.declare_dram_parameter("wstr", [P, ET], f32, isOutput=False)
    biasb = nc.declare_dram_parameter("biasb", [P, NB, F], f32, isOutput=False)
    out = nc.declare_dram_parameter("out", [NL, F], f32, isOutput=True)

    with tile.TileContext(nc) as tc:
        with (
            tc.tile_pool(name="hp", bufs=NPAIR) as hp,           # resident H pairs
            tc.tile_pool(name="smallp", bufs=1) as smallp,       # persistent small
            tc.tile_pool(name="rp", bufs=4) as rp,               # rowsum temps
            tc.tile_pool(name="gfp", bufs=2) as gfp,             # gpsimd fold out
            tc.tile_pool(name="ybp", bufs=1) as ybp,             # ybuf halves
            tc.tile_pool(name="y2rp", bufs=1) as y2rp,           # y2 raw staging
            tc.tile_pool(name="finp", bufs=2) as finp,           # out staging
            tc.tile_pool(name="ps_xw", bufs=1, space="PSUM") as ps_xw,
            tc.tile_pool(name="ps_y", bufs=2, space="PSUM") as ps_y,
            tc.tile_pool(name="ps_o", bufs=1, space="PSUM") as ps_o,
            tc.tile_pool(name="dramp", bufs=1, space="DRAM") as dramp,
        ):
            # ---- persistent small tensors ----
            xt_sb = smallp.tile([F, NL], f32, tag="xt")
            nc.sync.dma_start(xt_sb[:], xt[:, :])
            wmat_sb = smallp.tile([F, F], f32, tag="wmat")
            nc.sync.dma_start(wmat_sb[:], wmat[:, :])
            wstr_sb = smallp.tile([P, ET], f32, tag="wstr")
            nc.sync.dma_start(wstr_sb[:], wstr[:, :])
            bias_sb = smallp.tile([P, NB, F], f32, tag="bias")
            nc.sync.dma_start(bias_sb[:], biasb[:, :, :])
            dv_all = smallp.tile([P, NT], f32, tag="dv")
            dvf = smallp.tile([P, NT], f32, tag="dvf")
            r0_all = smallp.tile([P, NT], f32, tag="r0a")
            r1_all = smallp.tile([P, NT], f32, tag="r1a")
            r2_all = smallp.tile([P, NT], f32, tag="r2a")
            xs_sb = smallp.tile([P, NPAIR, 2, XPAD], f8, tag="xs")
            nc.gpsimd.memset(xs_sb[:, :, :, F:F + 1], 1.0)
            y2_sb = smallp.tile([P, EPAIR, 2, XPAD], f8, tag="y2")
            dumA = smallp.tile([P, 2560], f8, tag="dumA")  # ACT rowsum sink
            ident = smallp.tile([F, F], f32, tag="ident")
            make_identity(nc, ident)

            # ---- xw = x @ weight (x64) for all n-tiles, kept in PSUM ----
            xw_all = ps_xw.tile([P, NT * F], f32, tag="xw")
            for t in range(NT):
                nc.tensor.matmul(
                    xw_all[:, t * F:(t + 1) * F],
                    lhsT=xt_sb[:, t * P:(t + 1) * P], rhs=wmat_sb[:],
                    start=True, stop=True,
                )

            h_tiles = []

            def prep_pair(s):
                h_s = hp.tile([P, 2, E], f8, tag="h")
                eng = nc.sync if s % 2 == 0 else nc.scalar
                eng.dma_start(h_s[:], h[s, :, :, :])
                h_tiles.append(h_s)
                for j in range(2):
                    t = 2 * s + j
                    # rowsum 3-lane split:
                    #  DVE direct fp8 reduce [0:1536)
                    #  ACT copy+accum [1536:4096)
                    #  gpsimd fp8-fold [4096:8192) -> f16 [P,2048] -> DVE
                    gfo = gfp.tile([P, 2048], f16, tag="gf")
                    nc.gpsimd.tensor_tensor(
                        out=gfo[:], in0=h_s[:, j, 4096:6144],
                        in1=h_s[:, j, 6144:8192], op=add,
                    )
                    nc.vector.tensor_reduce(
                        out=r0_all[:, t:t + 1], in_=h_s[:, j, 0:1536],
                        axis=X, op=add,
                    )
                    nc.scalar.activation(
                        out=dumA[:], in_=h_s[:, j, 1536:4096], func=Copy,
                        accum_out=r1_all[:, t:t + 1],
                    )
                    nc.vector.tensor_reduce(
                        out=r2_all[:, t:t + 1], in_=gfo[:], axis=X, op=add
                    )

            def finalize_group(g):
                # batched dv + xs for 8 tiles: one TT/TT/recip/sqrt instead
                # of per-tile [P,1] ops (those cost ~1us fixed overhead each)
                lo, hi = g * 8, g * 8 + 8
                rs = rp.tile([P, 8], f32, tag="rs")
                nc.vector.tensor_tensor(
                    out=rs[:], in0=r0_all[:, lo:hi], in1=r1_all[:, lo:hi], op=add
                )
                nc.vector.tensor_tensor(
                    out=rs[:], in0=rs[:], in1=r2_all[:, lo:hi], op=add
                )
                nc.vector.reciprocal(out=rs[:], in_=rs[:])
                nc.scalar.sqrt(out=dv_all[:, lo:hi], in_=rs[:])
                for t in range(lo, hi):
                    nc.scalar.activation(
                        out=xs_sb[:, t // 2, t % 2, 0:F],
                        in_=xw_all[:, t * F:(t + 1) * F],
                        func=Copy, scale=dv_all[:, t:t + 1],
                    )

            for s in range(G):
                prep_pair(s)
            finalize_group(0)

            # ---- pass 1 (DoubleRow chains, 2 rotating banks) ----
            ybufs = [ybp.tile([F + 1, HE], bf16, name=f"yb{q}") for q in range(2)]

            def p1_block(b, g):
                yps = ps_y.tile([F + 1, EBLK], f32, tag="yps")
                for i in range(G):
                    pr = g * G + i
                    nc.tensor.matmul(
                        yps[:], lhsT=xs_sb[:, pr, :, 0:F + 1],
                        rhs=h_tiles[pr][:, :, b * EBLK:(b + 1) * EBLK],
                        start=(i == 0), stop=(i == G - 1),
                        perf_mode=DR,
                    )
                return yps

            b_outs = []

            def kick_half(q):
                b_in = dramp.tile([F + 1, HE], bf16, name=f"bi{q}")
                b_out = dramp.tile(
                    [F + 1, HE], bf16, name=f"bo{q}", addr_space="Shared"
                )
                nc.scalar.dma_start(b_in[:], ybufs[q][:])
                nc.gpsimd.collective_compute(
                    "AllReduce",
                    add,
                    ins=[b_in[:].opt()],
                    outs=[b_out[:].opt()],
                    replica_groups=[list(range(NCORES))],
                )
                b_outs.append(b_out)

            def y2_prep(q):
                # y2 half prep: XBAR transpose + de/wde + scale
                b_out = b_outs[q]
                y2raw = y2rp.tile([P, HE // P, F], bf16, tag="y2r")
                nc.scalar.dma_start_transpose(y2raw[:], b_out[0:F, :])
                cs = smallp.tile([P, HE // P], bf16, name=f"cs{q}")
                nc.scalar.dma_start(
                    cs[:], b_out[F, :].rearrange("(o p) -> p o", p=P)
                )
                de_t = smallp.tile([P, HE // P], f32, name=f"de{q}")
                nc.vector.reciprocal(out=de_t[:], in_=cs[:])
                wde = smallp.tile([P, HE // P], f32, name=f"wde{q}")
                nc.vector.tensor_tensor(
                    out=wde[:], in0=de_t[:],
                    in1=wstr_sb[:, q * (HE // P):(q + 1) * (HE // P)], op=mult,
                )
                for c in range(HE // P):
                    o = q * (HE // P) + c
                    nc.scalar.activation(
                        out=y2_sb[:, o // 2, o % 2, 0:F], in_=y2raw[:, c, :],
                        func=Copy, scale=wde[:, c:c + 1],
                    )

            # phase A: group 0 over blocks 0..7 (prep of pairs 4..7 interleaves)
            for b in range(8):
                if b % 2 == 0:
                    prep_pair(G + b // 2)
                yps = p1_block(b, 0)
                nc.vector.tensor_copy(
                    out=ybufs[0][:, b * EBLK:(b + 1) * EBLK], in_=yps[:]
                )
            finalize_group(1)
            # phase B: group 1 over blocks 0..7 -> half 0 final, collective flies
            for b in range(8):
                yps = p1_block(b, 1)
                dst = ybufs[0][:, b * EBLK:(b + 1) * EBLK]
                nc.vector.tensor_tensor(out=dst, in0=dst, in1=yps[:], op=add)
            kick_half(0)
            # ht chunks are host-packed contiguous ([8, P, 4, 2, NL]) so each
            # load is one straight-run HWDGE DMA on the idle Sync queue.
            # Chunk c reuses hp slot c, whose last reader is phase C
            # (pairs 0-3) / phase D (pairs 4-7).
            chunks = []

            def load_chunks(lo, hi):
                for c in range(lo, hi):
                    ck = hp.tile([P, 4, 2, NL], f8, tag="h")
                    nc.sync.dma_start(ck[:], ht[c, :, :, :, :])
                    chunks.append(ck)

            # phase C/D: blocks 8..15 while half-0 AllReduce is in flight
            for b in range(8, NB_BLK):
                yps = p1_block(b, 0)
                nc.vector.tensor_copy(
                    out=ybufs[1][:, (b - 8) * EBLK:(b - 7) * EBLK], in_=yps[:]
                )
            load_chunks(0, 4)
            for b in range(8, NB_BLK):
                yps = p1_block(b, 1)
                dst = ybufs[1][:, (b - 8) * EBLK:(b - 7) * EBLK]
                nc.vector.tensor_tensor(out=dst, in0=dst, in1=yps[:], op=add)
            kick_half(1)
            load_chunks(4, 8)
            y2_prep(0)

            # ---- pass 2: out^T accumulation via DoubleRow over e-pairs ----
            # u=0..15 only needs y2 half 0 + chunks 0-3, so it overlaps the
            # half-1 AllReduce still in flight.
            o_tiles = [ps_o.tile([F, NBLK], f32, name=f"o{j}") for j in range(NB)]
            for u in range(EPAIR // 2):
                htt = chunks[u // 4]
                for j in range(NB):
                    nc.tensor.matmul(
                        o_tiles[j][:], lhsT=y2_sb[:, u, :, 0:F],
                        rhs=htt[:, u % 4, :, j * NBLK:(j + 1) * NBLK],
                        start=(u == 0), stop=False,
                        perf_mode=DR,
                    )
            y2_prep(1)
            for u in range(EPAIR // 2, EPAIR):
                htt = chunks[u // 4]
                for j in range(NB):
                    nc.tensor.matmul(
                        o_tiles[j][:], lhsT=y2_sb[:, u, :, 0:F],
                        rhs=htt[:, u % 4, :, j * NBLK:(j + 1) * NBLK],
                        start=False, stop=(u == EPAIR - 1),
                        perf_mode=DR,
                    )

            # ---- finish: transpose back, dv/1024 scale, bias, store ----
            nc.vector.tensor_scalar_mul(dvf[:], dv_all[:], 1.0 / 1024.0)
            for j in range(NB):
                s1 = finp.tile([F, NBLK], f32, tag="s1")
                nc.vector.tensor_copy(out=s1[:], in_=o_tiles[j][:])
                osb = finp.tile([P, NBLK // P, F], f32, tag="osb")
                for c in range(NBLK // P):
                    tp = ps_y.tile([P, F], f32, tag="yps")
                    nc.tensor.transpose(
                        tp[:], s1[:, c * P:(c + 1) * P], ident[:]
                    )
                    nc.scalar.activation(
                        out=osb[:, c, :], in_=tp[:], func=Copy,
                        scale=dvf[:, j * (NBLK // P) + c:j * (NBLK // P) + c + 1],
                    )
                nc.vector.tensor_tensor(
                    out=osb[:], in0=osb[:], in1=bias_sb[:], op=add
                )
                nc.scalar.dma_start(
                    out[j * NBLK:(j + 1) * NBLK, :].rearrange(
                        "(c p) f -> p c f", p=P
                    ),
                    osb[:],
                )

    nc.finalize()
    return nc


def _get_program():
    if "nc" not in _prog_cache:
        _prog_cache["nc"] = _build_program()
    return _prog_cache["nc"]


def make_in_maps(x, H, W, weight, bias):
    x = np.asarray(x, dtype=np.float32)
    H = np.asarray(H, dtype=np.float32)
    W = np.asarray(W, dtype=np.float32)
    weight = np.asarray(weight, dtype=np.float32)
    bias = np.asarray(bias, dtype=np.float32)

    H_f8 = H.astype(ml_dtypes.float8_e4m3)
    wstr = np.ascontiguousarray((16.0 * W).reshape(ET, P).T.astype(np.float32))
    biasb = np.ascontiguousarray(
        np.tile(bias[None, None, :], (P, NB, 1)).astype(np.float32)
    )
    wmat = np.ascontiguousarray(64.0 * weight)

    in_maps = []
    for c in range(NCORES):
        hs = H_f8[c * NL:(c + 1) * NL, :]
        # natural pairs: [NPAIR, P, 2, E], (s, p, j) -> row 256 s + 128 j + p
        hpair = np.ascontiguousarray(
            hs.reshape(NPAIR, 2, P, E).transpose(0, 2, 1, 3)
        )
        # transposed pairs, chunk-contiguous: [8, P, 4, 2, NL],
        # (c, p, u4, j) -> col 256 (4c + u4) + 128 j + p
        htpair = np.ascontiguousarray(
            hs.T.reshape(8, 4, 2, P, NL).transpose(0, 3, 1, 2, 4)
        )
        in_maps.append({
            "h": hpair,
            "ht": htpair,
            "xt": np.ascontiguousarray(x[c * NL:(c + 1) * NL, :].T),
            "wmat": wmat,
            "wstr": wstr,
            "biasb": biasb,
        })
    return in_maps


def run(x, H, W, weight, bias, trace=False, **kw):
    from concourse.bass_utils import run_bass_kernel_spmd

    nc = _get_program()
    in_maps = make_in_maps(x, H, W, weight, bias)
    res = run_bass_kernel_spmd(nc, in_maps, list(range(NCORES)), trace=trace, **kw)
    out = np.concatenate(
        [res.results[c]["out"] for c in range(NCORES)], axis=0
    ).astype(np.float32)
    return out, res


def kernel(x, H, W, weight, bias):
    out, _ = run(x, H, W, weight, bias, trace=False)
    return out

